# revision 23
# baseline (speedup 1.0000x reference)
"""Trainium2 Bass kernel for GQA attention (B=4, S=1024, D=4096, HQ=32, HKV=8).

Sharding: 8 cores = 4 batches x 2 head-groups. Each core computes one batch
with 16 q-heads / 4 kv-heads (Wq/Wk/Wv column-sharded, Wo row-sharded). The
two head-group partial outputs per batch (bf16) are summed on the host, then
transposed (device emits out^T [Dout, S]) and bias bo added.

Device dataflow per core (all matmuls bf16):
  P1 (QKV) runs as 24 single-block column matmuls ordered [k,v,q0..q3] per
  kv group so attention for head h starts as soon as its q block is done.
  x streams in as two half-column waves over 4 DMA queues; w0/cd/bias ride
  the tensor-engine queue so the HBM-bound startup has no queue conflicts.
  P2 (attention) is interleaved into P1: per q block, emit scores
  (kT-stationary, already-transposed scoresT), exp on ScalarE (unsafe
  softmax), causal mask on the diagonal, then the PREVIOUS head's
  av+normalize so the scalar-engine exp of head h overlaps the PE work of
  head h+1 and the remaining P1 blocks. av uses aT-stationary matmuls with
  a 129-wide moving operand [v | ones] so the softmax denominator lands in
  psum column 128 of the same accumulation group (no separate den matmuls).
  Normalization is a per-chunk reciprocal + tensor_scalar multiply; oT for
  P3 is produced by XBAR dma transposes (gpsimd queue), except the last two
  heads which use PE transposes so P3's tail isn't blocked on the XBAR.
  The rope half-swap runs as partition-offset DVE multiplies (no PE time);
  v is transposed into [s2, d] layout by XBAR dma transposes.
  P3 (Wo): out[m-chunk, s] accumulated over 16 head-chunks with
  Wo-row-stationary matmuls; Wo streamed as 2x half DMAs per m-chunk on
  the gpsimd+scalar queues; per-block qkv weights alternate sync/vector
  queues. Output is written bf16 (host sums partials in f32); the last
  m-chunk evicts lo/hi halves sequentially so the final DMA overlaps PE.
"""

import math
import os

import numpy as np
import ml_dtypes

import concourse.bass as bass
import concourse.mybir as mybir
import concourse.tile as tile
from concourse import bacc
from concourse.bass_utils import run_bass_kernel_spmd
from concourse.masks import make_identity

B, S, D = 4, 1024, 4096
HQ, HKV, HD = 32, 8, 128
NH = 16          # q heads per core
NKV = 4          # kv heads per core
DQ = NH * HD     # 2048
DK = NKV * HD    # 512
NDC = D // 128   # 32 D-chunks
NSC = S // 128   # 8 s-chunks
QK_SCALE = 1.0 / math.sqrt(HD)

F32 = mybir.dt.float32
BF16 = mybir.dt.bfloat16

_GRAPH_CACHE = {}
LAST_PROFILE = None


def _block_specs():
    """24 P1 blocks in emission order: per kv group [k, v, q0..q3]."""
    blocks = []
    for g4 in range(NKV):
        blocks.append(("k", g4))
        blocks.append(("v", g4))
        for i in range(4):
            blocks.append(("q", 4 * g4 + i))
    return blocks


def _score_chunks(j):
    """Global (c0, c1) column chunks for k-chunk j (causal: s1 >= j*128),
    split at the 512 boundary so each chunk fits one psum bank."""
    if j < 4:
        return [(j * 128, 512), (512, 1024)]
    return [(j * 128, 1024)]


def _build_graph():
    nc = bacc.Bacc(debug=False)

    xt_ext = nc.dram_tensor("xt", [NDC, 128, S], BF16, kind="ExternalInput")
    # per-block weight columns, dc-major rows: [24, 128, NDC*128]
    wqkv_ext = nc.dram_tensor("wqkv", [24, 128, NDC * 128], BF16,
                              kind="ExternalInput")
    # Wo row-shard packed per output m-chunk: [32, 128, NH*128]
    wo_ext = nc.dram_tensor("wo", [NDC, 128, NH * 128], BF16,
                            kind="ExternalInput")
    cd1_ext = nc.dram_tensor("cd1", [128, S], BF16, kind="ExternalInput")
    cd2_ext = nc.dram_tensor("cd2", [128, S], BF16, kind="ExternalInput")
    # bias column tile: col bi = bias for block bi
    bqkv_ext = nc.dram_tensor("bqkv", [128, 24], F32, kind="ExternalInput")
    out_ext = nc.dram_tensor("out", [D, S], BF16, kind="ExternalOutput")
    debug = bool(os.environ.get("BASS_DEBUG_TAPS"))
    dbg_exts = {}
    if debug:
        dbg_exts["dbg_qT"] = nc.dram_tensor("dbg_qT", [128, NH * S], F32,
                                            kind="ExternalOutput")
        dbg_exts["dbg_kT"] = nc.dram_tensor("dbg_kT", [128, NKV * S], F32,
                                            kind="ExternalOutput")
        dbg_exts["dbg_v"] = nc.dram_tensor("dbg_v", [128, NSC * NKV * 129],
                                           F32, kind="ExternalOutput")
        dbg_exts["dbg_oT"] = nc.dram_tensor("dbg_oT", [128, NH * S], F32,
                                            kind="ExternalOutput")

    with tile.TileContext(nc) as tc:
        cpool = tc.alloc_tile_pool(name="const", bufs=1)
        ppool = tc.alloc_tile_pool(name="persist", bufs=1)
        wpool = tc.alloc_tile_pool(name="wk", bufs=1)
        xpool = tc.alloc_tile_pool(name="xts", bufs=1)
        psW = tc.alloc_tile_pool(name="psW", bufs=5, space="PSUM")
        psO = tc.alloc_tile_pool(name="psO", bufs=1, space="PSUM")

        # ---- constants (engine setup ops are emitted after the startup
        # DMA issues so gpsimd's first weight DMAs trigger immediately) ----
        maskT = cpool.tile([128, 128], BF16)   # 1 where s1 >= s2 else 0
        ident_b = cpool.tile([128, 128], BF16)
        cd1_sb = cpool.tile([128, S], BF16)
        cd2_sb = cpool.tile([128, S], BF16)
        bias_sb = cpool.tile([128, 24], F32)

        # ---- persistent activations ----
        qT_all = ppool.tile([128, NH * S], BF16)    # [d, h*S + s]
        kT_all = ppool.tile([128, NKV * S], BF16)   # [d, g4*S + s]
        # v in [s2, d] layout + ones col for the denominator:
        # v_all[:, sc, g4, 0:128] = v chunk, [..., 128] = 1.0
        v_all = ppool.tile([128, NSC, NKV, 129], BF16)
        oT_all = ppool.tile([128, NH * S], BF16)    # [d, h*S + s]

        xts = [
            xpool.tile([128, S], BF16, tag=f"xt{dc}", name=f"xt{dc}")
            for dc in range(NDC)
        ]
        w0a = wpool.tile([128, 2048], BF16, tag="w", bufs=3, name="w_0a")
        w0b = wpool.tile([128, 2048], BF16, tag="w", bufs=3, name="w_0b")
        # startup: x streams as two half-column waves round-robin over the 3
        # DMA engines (arrival tracks the dc consumption order); block-0
        # weight pieces lead on sync/scalar/gpsimd at their consumption
        # deadlines, rope/bias constants fill in on scalar
        xq = [nc.sync, nc.scalar, nc.gpsimd]
        nc.sync.dma_start(out=w0a[:, 0:512], in_=wqkv_ext[0, :, 0:512])
        nc.scalar.dma_start(out=bias_sb[:], in_=bqkv_ext[:])
        nc.gpsimd.dma_start(out=w0a[:, 512:2048],
                            in_=wqkv_ext[0, :, 512:2048])
        nc.gpsimd.dma_start(out=w0b[:], in_=wqkv_ext[0, :, 2048:4096])
        for dc in range(NDC):
            xq[(dc + 1) % 3].dma_start(out=xts[dc][:, 0:512],
                                       in_=xt_ext[dc, :, 0:512])
            if dc == 10:
                nc.scalar.dma_start(out=cd1_sb[:], in_=cd1_ext[:])
            if dc == 13:
                nc.scalar.dma_start(out=cd2_sb[:], in_=cd2_ext[:])
        for dc in range(NDC):
            xq[(dc + 1) % 3].dma_start(out=xts[dc][:, 512:1024],
                                       in_=xt_ext[dc, :, 512:1024])

        # deferred constant setup (consumers start several blocks in, and
        # emitting these late keeps gpsimd's first weight DMAs unblocked)
        nc.gpsimd.memset(maskT[:], 1.0)
        nc.gpsimd.affine_select(
            out=maskT[:], in_=maskT[:], compare_op=mybir.AluOpType.is_ge,
            fill=0.0, base=0, pattern=[[1, 128]], channel_multiplier=-1)
        make_identity(nc, ident_b)
        nc.gpsimd.memset(v_all[:, :, :, 128:129], 1.0)

        w_tiles = {0: (w0a, w0b)}

        def ensure_w(bi):
            # issue block bi's weight DMAs one block ahead of use so the
            # sync queue's trigger is already in flight before any
            # attention-coupled wait (oT transpose) enters the sync stream
            if bi in w_tiles or bi >= 24:
                return
            w_a = wpool.tile([128, 2048], BF16, tag="w", bufs=3,
                             name=f"w{bi}a")
            nc.sync.dma_start(out=w_a[:], in_=wqkv_ext[bi, :, 0:2048])
            w_b = wpool.tile([128, 2048], BF16, tag="w", bufs=3,
                             name=f"w{bi}b")
            nc.gpsimd.dma_start(out=w_b[:], in_=wqkv_ext[bi, :, 2048:4096])
            w_tiles[bi] = (w_a, w_b)

        def emit_p1_block(bi, kind, idx):
            """Emits the block matmuls and the half-0 evict/rope chain.
            Returns a closure emitting the half-1 rope tail (q/k blocks),
            which the caller places where the PE has other work to hide
            the evict latency; v blocks return their second transpose set."""
            ensure_w(bi)
            w_a, w_b = w_tiles.pop(bi)
            ensure_w(bi + 1)
            acc0 = psW.tile([128, 512], F32, tag="acc", bufs=2,
                            name=f"acc0_{bi}")
            acc1 = psW.tile([128, 512], F32, tag="acc", bufs=2,
                            name=f"acc1_{bi}")
            tf = wpool.tile([128, S], BF16, tag="tf", bufs=2, name=f"tf{bi}")
            if kind != "v":
                dstT, col = (kT_all, idx) if kind == "k" else (qT_all, idx)
                t1 = wpool.tile([128, S], BF16, tag="rt0", bufs=2,
                                name=f"rt0_{bi}")

            def evict(hf, acc):
                nc.vector.tensor_scalar(
                    out=tf[:, hf * 512:(hf + 1) * 512], in0=acc[:],
                    scalar1=bias_sb[:, bi:bi + 1], scalar2=None,
                    op0=mybir.AluOpType.add)

            def rope(hf):
                # complex multiply via partition-offset DVE ops: the
                # half-swap writes the product of tf's half and cd2s (cd2
                # with halves pre-swapped on the host, so both INPUTS share
                # a partition base as the ISA requires) to the opposite
                # 64-partition half of t2
                cs = slice(hf * 512, (hf + 1) * 512)
                t2 = wpool.tile([128, 512], BF16, tag="rt1", bufs=2,
                                name=f"rt1_{bi}_{hf}")
                nc.vector.tensor_tensor(
                    out=t1[:, cs], in0=tf[:, cs], in1=cd1_sb[:, cs],
                    op=mybir.AluOpType.mult)
                nc.vector.tensor_tensor(
                    out=t2[0:64, :], in0=tf[64:128, cs],
                    in1=cd2_sb[64:128, cs], op=mybir.AluOpType.mult)
                nc.vector.tensor_tensor(
                    out=t2[64:128, :], in0=tf[0:64, cs],
                    in1=cd2_sb[0:64, cs], op=mybir.AluOpType.mult)
                nc.vector.tensor_tensor(
                    out=dstT[:, col * S + hf * 512:col * S + (hf + 1) * 512],
                    in0=t1[:, cs], in1=t2[:], op=mybir.AluOpType.add)

            def transposes(scs):
                for sc in scs:
                    tp = psW.tile([128, 128], BF16, tag="sc", bufs=3,
                                  name=f"tp{bi}_{sc}")
                    nc.tensor.transpose(
                        tp[:], tf[:, sc * 128:(sc + 1) * 128], ident_b)
                    nc.scalar.copy(v_all[:, sc, idx, 0:128], tp[:])

            # half 0 accumulates fully first so its evict/rope chain runs
            # during half 1's matmuls
            def wslice(dc):
                t = w_a if dc < 16 else w_b
                return t[:, (dc % 16) * 128:(dc % 16 + 1) * 128]

            for dc in range(NDC):
                nc.tensor.matmul(acc0[:], wslice(dc), xts[dc][:, 0:512],
                                 start=(dc == 0), stop=(dc == NDC - 1))
            evict(0, acc0)
            for dc in range(NDC):
                nc.tensor.matmul(acc1[:], wslice(dc), xts[dc][:, 512:1024],
                                 start=(dc == 0), stop=(dc == NDC - 1))
                if dc == 6 and kind != "v":
                    rope(0)
                if dc == 20 and kind == "v":
                    transposes(range(4))
            evict(1, acc1)
            if kind == "v":
                return lambda: transposes(range(4, NSC))
            return lambda: rope(1)

        def emit_scores_exps(h):
            """scoresT_j = kT_j.T @ qT per 512-chunk (half-0 column chunks
            first: their qT rope half finishes earlier); exp -> aT; causal
            mask on the diagonal 128-block (gpsimd). Returns
            {(j, ci): (aT tile, c0)}."""
            g4 = h // 4
            chunks = ([(j, 0) for j in range(4)] + [(j, 1) for j in range(4)]
                      + [(j, 0) for j in range(4, NSC)])
            aT = {}
            for j, ci in chunks:
                c0, c1 = _score_chunks(j)[ci]
                w = c1 - c0
                scp = psW.tile([128, w], F32, tag="sc", bufs=3,
                               name=f"scp{h}_{j}_{ci}")
                nc.tensor.matmul(
                    scp[:], kT_all[:, g4 * S + j * 128:g4 * S + (j + 1) * 128],
                    qT_all[:, h * S + c0:h * S + c1],
                    start=True, stop=True)
                a = wpool.tile([128, w], BF16, tag="aT", bufs=12,
                               name=f"aT{h}_{j}_{ci}")
                nc.scalar.activation(
                    a[:], scp[:], mybir.ActivationFunctionType.Exp,
                    scale=QK_SCALE)
                if ci == 0:
                    # causal mask on diagonal block (local cols 0:128)
                    nc.gpsimd.tensor_tensor(
                        out=a[:, 0:128], in0=a[:, 0:128], in1=maskT[:],
                        op=mybir.AluOpType.mult)
                aT[(j, ci)] = (a, c0)
            return aT

        def emit_av(h, aT, pe_transpose=False):
            """o[s1c, d|den] += aT_j[:, s1c].T @ [v_j | ones] — the 129th
            moving column lands the softmax denominator in the same psum
            accumulation group. Normalize per chunk (reciprocal +
            tensor_scalar), then transpose into oT_all (XBAR dma; PE
            matmul-transpose for the last heads so P3 isn't XBAR-blocked)."""
            g4 = h // 4
            # 8 groups of 129 f32, packed 3 per psum bank (3*129 <= 512)
            o_ps = psO.tile([128, 3, 512], F32, tag="o", bufs=1,
                            name=f"o_{h}")
            rcp = wpool.tile([128, NSC], F32, tag="rcp", bufs=2,
                             name=f"rcp{h}")
            o_sb = wpool.tile([128, NSC, 128], BF16, tag="osb", bufs=2,
                              name=f"osb{h}")
            def normalize(s1cs):
                # runs once a bank's groups all closed, so the DVE never
                # reads a psum bank the PE is still accumulating into
                for s1c in s1cs:
                    bank, off = divmod(s1c, 3)
                    off *= 129
                    nc.vector.reciprocal(rcp[:, s1c:s1c + 1],
                                         o_ps[:, bank, off + 128:off + 129])
                    nc.vector.tensor_scalar(
                        out=o_sb[:, s1c, :],
                        in0=o_ps[:, bank, off:off + 128],
                        scalar1=rcp[:, s1c:s1c + 1], scalar2=None,
                        op0=mybir.AluOpType.mult)
                    if pe_transpose:
                        dst = oT_all[:, h * S + s1c * 128:
                                     h * S + (s1c + 1) * 128]
                        tp = psW.tile([128, 128], BF16, tag="sc", bufs=3,
                                      name=f"tp_o{h}_{s1c}")
                        nc.tensor.transpose(tp[:], o_sb[:, s1c, :], ident_b)
                        nc.scalar.copy(dst, tp[:])

            def flush_oT():
                # XBAR transposes split across the two HWDGE queues; the
                # caller emits this AFTER the next head's exps so neither
                # queue has latency-critical work sitting behind the waits
                for s1c in range(NSC):
                    dst = oT_all[:, h * S + s1c * 128:h * S + (s1c + 1) * 128]
                    eng = nc.sync if s1c % 2 else nc.scalar
                    eng.dma_start(out=dst, in_=o_sb[:, s1c, :],
                                  transpose=True)

            for s1c in range(NSC):
                bank, off = divmod(s1c, 3)
                off *= 129
                for j in range(s1c + 1):
                    # find chunk holding global col s1c*128
                    if j < 4 and s1c >= 4:
                        a, c0 = aT[(j, 1)]
                    else:
                        a, c0 = aT[(j, 0)]
                    loc = s1c * 128 - c0
                    nc.tensor.matmul(
                        o_ps[:, bank, off:off + 129], a[:, loc:loc + 128],
                        v_all[:, j, g4, :],
                        start=(j == 0), stop=(j == s1c),
                        skip_group_check=True)
                if s1c == 2:
                    normalize((0, 1, 2))
                elif s1c == 5:
                    normalize((3, 4, 5))
            normalize((6, 7))
            return None if pe_transpose else flush_oT

        def fetch_wo(m):
            wo_m = wpool.tile([128, NH, 128], BF16, tag="wo", bufs=3,
                              name=f"wo{m}")
            nc.gpsimd.dma_start(out=wo_m[:, 0:8, :], in_=wo_ext[m, :, 0:1024])
            nc.scalar.dma_start(out=wo_m[:, 8:16, :],
                                in_=wo_ext[m, :, 1024:2048])
            return wo_m

        # ---------------- P1 + P2 interleaved ----------------
        pending = None   # (head, aT dict) awaiting av emission
        deferred = None  # half-1 rope tail of the previous k block
        for bi, (kind, idx) in enumerate(_block_specs()):
            tail1 = emit_p1_block(bi, kind, idx)
            if deferred is not None:
                deferred()
                deferred = None
            if kind == "q":
                flush_prev = None
                if pending is not None:
                    flush_prev = emit_av(*pending,
                                         pe_transpose=(pending[0] >= NH - 2))
                tail1()  # q half-1 rope must precede its own scores
                pending = (idx, emit_scores_exps(idx))
                if flush_prev is not None:
                    flush_prev()
            elif tail1 is not None:
                deferred = tail1
        wo_tiles = {m: fetch_wo(m) for m in range(3)}

        # ---------------- P3: Wo ----------------
        # P3 accumulators live in the P1 "acc" psum ring (no release
        # barrier); m=0's first 15 head-chunks are emitted BEFORE the last
        # head's av so the PE chews them while scalar runs head 15's exps.
        def p3_setup(m):
            wo_m = wo_tiles.pop(m) if m in wo_tiles else fetch_wo(m)
            acc_lo = psW.tile([128, 512], F32, tag="acc", bufs=2,
                              name=f"wp{m}_lo")
            acc_hi = psW.tile([128, 512], F32, tag="acc", bufs=2,
                              name=f"wp{m}_hi")
            return wo_m, acc_lo, acc_hi

        def p3_mms(st, c_from, c_to):
            wo_m, acc_lo, acc_hi = st
            for c in range(c_from, c_to):
                lhs = wo_m[:, c, :]
                nc.tensor.matmul(acc_lo[:], lhs,
                                 oT_all[:, c * S:c * S + 512],
                                 start=(c == 0), stop=(c == NH - 1))
                nc.tensor.matmul(acc_hi[:], lhs,
                                 oT_all[:, c * S + 512:c * S + 1024],
                                 start=(c == 0), stop=(c == NH - 1))

        def p3_finish(m, st):
            _, acc_lo, acc_hi = st
            oev = wpool.tile([128, S], BF16, tag="oev", bufs=2,
                             name=f"oev{m}")
            nc.scalar.copy(oev[:, 0:512], acc_lo[:])
            nc.scalar.copy(oev[:, 512:1024], acc_hi[:])
            nc.sync.dma_start(
                out=out_ext[m * 128:(m + 1) * 128, :], in_=oev[:])

        st0 = p3_setup(0)
        p3_mms(st0, 0, NH - 1)
        emit_av(*pending, pe_transpose=True)
        p3_mms(st0, NH - 1, NH)
        p3_finish(0, st0)
        # m=1 also rides the acc ring so the pool-release barrier below
        # resolves while its c-loop runs
        st1 = p3_setup(1)
        p3_mms(st1, 0, NH)
        p3_finish(1, st1)

        psO.release()
        psW.release()
        xpool.release()
        psP3 = tc.alloc_tile_pool(name="psP3", bufs=2, space="PSUM")
        for m in range(2, NDC):
            wo_m = wo_tiles.pop(m) if m in wo_tiles else fetch_wo(m)
            acc = psP3.tile([128, S], F32, tag="wps", bufs=2, name=f"wp{m}")
            oev = wpool.tile([128, S], BF16, tag="oev", bufs=2,
                             name=f"oev{m}")
            if m < NDC - 1:
                for c in range(NH):
                    lhs = wo_m[:, c, :]
                    nc.tensor.matmul(acc[:, 0:512], lhs,
                                     oT_all[:, c * S:c * S + 512],
                                     start=(c == 0), stop=(c == NH - 1))
                    nc.tensor.matmul(acc[:, 512:1024], lhs,
                                     oT_all[:, c * S + 512:c * S + 1024],
                                     start=(c == 0), stop=(c == NH - 1))
                nc.scalar.copy(oev[:], acc[:])
                nc.sync.dma_start(
                    out=out_ext[m * 128:(m + 1) * 128, :], in_=oev[:])
            else:
                # last m-chunk: finish + ship the lo half while the PE runs
                # the hi half so the final out DMA overlaps compute
                for half in range(2):
                    cs = slice(half * 512, (half + 1) * 512)
                    for c in range(NH):
                        nc.tensor.matmul(acc[:, cs], wo_m[:, c, :],
                                         oT_all[:, c * S + half * 512:
                                                c * S + (half + 1) * 512],
                                         start=(c == 0), stop=(c == NH - 1))
                    nc.scalar.copy(oev[:, cs], acc[:, cs])
                    nc.sync.dma_start(
                        out=out_ext[m * 128:(m + 1) * 128,
                                    half * 512:(half + 1) * 512],
                        in_=oev[:, cs])
        psP3.release()

        if debug:
            for nm, t in [("dbg_qT", qT_all), ("dbg_kT", kT_all),
                          ("dbg_v", v_all), ("dbg_oT", oT_all)]:
                nc.gpsimd.dma_start(out=dbg_exts[nm][:], in_=t[:])

        cpool.seal()
        ppool.seal()
        wpool.seal()

    nc.compile()
    return nc


def _ev(base):
    return np.concatenate([np.arange(base, base + HD, 2),
                           np.arange(base + 1, base + HD, 2)])


def _pack_wblock(Wcols):
    # [D, 128] -> [128, NDC*128] with dc-major columns
    return Wcols.reshape(NDC, 128, 128).transpose(1, 0, 2).reshape(128, -1)


def kernel(x, freqs_cis, Wq, bq, Wk, bk, Wv, bv, Wo, bo, startpos):
    global LAST_PROFILE
    x = np.asarray(x, dtype=np.float32)
    freqs_cis = np.asarray(freqs_cis, dtype=np.float32)
    Wq = np.asarray(Wq, dtype=np.float32)
    Wk = np.asarray(Wk, dtype=np.float32)
    Wv = np.asarray(Wv, dtype=np.float32)
    Wo = np.asarray(Wo, dtype=np.float32)
    bq = np.asarray(bq, dtype=np.float32)
    bk = np.asarray(bk, dtype=np.float32)
    bv = np.asarray(bv, dtype=np.float32)
    bo = np.asarray(bo, dtype=np.float32)
    assert int(startpos) == 0

    bf = lambda a: np.ascontiguousarray(a.astype(ml_dtypes.bfloat16))
    f32c = lambda a: np.ascontiguousarray(a.astype(np.float32))

    # rope coefficients in [d, s] layout: C64[i, s] = fc[s, i, 0]
    C64 = freqs_cis[:, :, 0].T          # [64, S]
    D64 = freqs_cis[:, :, 1].T
    cd1 = bf(np.vstack([C64, C64]))
    # halves pre-swapped: row i<64 multiplies tf's half a (writing half b),
    # row i>=64 multiplies tf's half b (writing half a)
    cd2 = bf(np.vstack([D64, -D64]))

    in_maps = []
    for core in range(8):
        b, g = core // 2, core % 2
        if core < 2:  # weight shards depend only on g; reuse for later cores
            wblocks, bcols = [], []
            for kind, idx in _block_specs():
                if kind == "k":
                    sel = _ev((g * NKV + idx) * HD)
                    wblocks.append(_pack_wblock(Wk[:, sel]))
                    bcols.append(bk[sel])
                elif kind == "v":
                    base = (g * NKV + idx) * HD
                    sel = np.arange(base, base + HD)
                    wblocks.append(_pack_wblock(Wv[:, sel]))
                    bcols.append(bv[sel])
                else:
                    sel = _ev((g * NH + idx) * HD)
                    wblocks.append(_pack_wblock(Wq[:, sel]))
                    bcols.append(bq[sel])
            wqkv_h = bf(np.stack(wblocks))                  # [24, 128, 4096]
            bqkv = f32c(np.stack(bcols, axis=1))            # [128, 24]
            Wos = Wo[g * DQ:(g + 1) * DQ, :]                # [2048, 4096]
            wo_h = bf(np.stack([
                Wos[:, m * 128:(m + 1) * 128]
                .reshape(NH, 128, 128).transpose(1, 0, 2).reshape(128, -1)
                for m in range(NDC)
            ]))                                             # [32, 128, 2048]
        else:
            prev = in_maps[core - 2]
            wqkv_h, wo_h, bqkv = prev["wqkv"], prev["wo"], prev["bqkv"]
        xt_h = bf(x[b].T.reshape(NDC, 128, S))
        in_maps.append({
            "xt": xt_h, "wqkv": wqkv_h, "wo": wo_h,
            "cd1": cd1, "cd2": cd2, "bqkv": bqkv,
        })

    if "nc" not in _GRAPH_CACHE:
        _GRAPH_CACHE["nc"] = _build_graph()
    nc = _GRAPH_CACHE["nc"]

    res = run_bass_kernel_spmd(
        nc, in_maps, core_ids=list(range(8)),
        trace=bool(os.environ.get("BASS_TRACE")))
    LAST_PROFILE = res

    out = np.empty((B, S, D), dtype=np.float32)
    for b in range(B):
        t = (res.results[2 * b]["out"].astype(np.float32)
             + res.results[2 * b + 1]["out"].astype(np.float32))
        out[b] = t.T + bo[None, :]
    return out


# revision 28
# speedup vs baseline: 1.0169x; 1.0169x over previous
"""Trainium2 Bass kernel for GQA attention (B=4, S=1024, D=4096, HQ=32, HKV=8).

Sharding: 8 cores = 4 batches x 2 head-groups. Each core computes one batch
with 16 q-heads / 4 kv-heads (Wq/Wk/Wv column-sharded, Wo row-sharded). The
two head-group partial outputs per batch (bf16) are summed on the host, then
transposed (device emits out^T [Dout, S]) and bias bo added.

Device dataflow per core (all matmuls bf16):
  P1 (QKV) runs as 24 single-block column matmuls ordered [k,v,q0..q3] per
  kv group so attention for head h starts as soon as its q block is done.
  x streams in as two half-column waves over 4 DMA queues; w0/cd/bias ride
  the tensor-engine queue so the HBM-bound startup has no queue conflicts.
  P2 (attention) is interleaved into P1: per q block, emit scores
  (kT-stationary, already-transposed scoresT), exp on ScalarE (unsafe
  softmax), causal mask on the diagonal, then the PREVIOUS head's
  av+normalize so the scalar-engine exp of head h overlaps the PE work of
  head h+1 and the remaining P1 blocks. av uses aT-stationary matmuls with
  a 129-wide moving operand [v | ones] so the softmax denominator lands in
  psum column 128 of the same accumulation group (no separate den matmuls).
  Normalization is a per-chunk reciprocal + tensor_scalar multiply; oT for
  P3 is produced by XBAR dma transposes (gpsimd queue), except the last two
  heads which use PE transposes so P3's tail isn't blocked on the XBAR.
  The rope half-swap runs as partition-offset DVE multiplies (no PE time);
  v is transposed into [s2, d] layout by XBAR dma transposes.
  P3 (Wo): out[m-chunk, s] accumulated over 16 head-chunks with
  Wo-row-stationary matmuls; Wo streamed as 2x half DMAs per m-chunk on
  the gpsimd+scalar queues; per-block qkv weights alternate sync/vector
  queues. Output is written bf16 (host sums partials in f32); the last
  m-chunk evicts lo/hi halves sequentially so the final DMA overlaps PE.
"""

import math
import os

import numpy as np
import ml_dtypes

import concourse.bass as bass
import concourse.mybir as mybir
import concourse.tile as tile
from concourse import bacc
from concourse.bass_utils import run_bass_kernel_spmd
from concourse.masks import make_identity

B, S, D = 4, 1024, 4096
HQ, HKV, HD = 32, 8, 128
NH = 16          # q heads per core
NKV = 4          # kv heads per core
DQ = NH * HD     # 2048
DK = NKV * HD    # 512
NDC = D // 128   # 32 D-chunks
NSC = S // 128   # 8 s-chunks
QK_SCALE = 1.0 / math.sqrt(HD)

F32 = mybir.dt.float32
BF16 = mybir.dt.bfloat16

_GRAPH_CACHE = {}
LAST_PROFILE = None


def _block_specs():
    """24 P1 blocks in emission order: per kv group [k, v, q0..q3]."""
    blocks = []
    for g4 in range(NKV):
        blocks.append(("k", g4))
        blocks.append(("v", g4))
        for i in range(4):
            blocks.append(("q", 4 * g4 + i))
    return blocks


def _score_chunks(j):
    """Global (c0, c1) column chunks for k-chunk j (causal: s1 >= j*128),
    split at the 512 boundary so each chunk fits one psum bank."""
    if j < 4:
        return [(j * 128, 512), (512, 1024)]
    return [(j * 128, 1024)]


def _build_graph():
    nc = bacc.Bacc(debug=False)

    xt_ext = nc.dram_tensor("xt", [NDC, 128, S], BF16, kind="ExternalInput")
    # per-block weight columns, dc-major rows: [24, 128, NDC*128]
    wqkv_ext = nc.dram_tensor("wqkv", [24, 128, NDC * 128], BF16,
                              kind="ExternalInput")
    # Wo row-shard packed per output m-chunk: [32, 128, NH*128]
    wo_ext = nc.dram_tensor("wo", [NDC, 128, NH * 128], BF16,
                            kind="ExternalInput")
    cd1_ext = nc.dram_tensor("cd1", [128, S], BF16, kind="ExternalInput")
    cd2_ext = nc.dram_tensor("cd2", [128, S], BF16, kind="ExternalInput")
    # bias column tile: col bi = bias for block bi
    bqkv_ext = nc.dram_tensor("bqkv", [128, 24], F32, kind="ExternalInput")
    out_ext = nc.dram_tensor("out", [D, S], BF16, kind="ExternalOutput")
    debug = bool(os.environ.get("BASS_DEBUG_TAPS"))
    dbg_exts = {}
    if debug:
        dbg_exts["dbg_qT"] = nc.dram_tensor("dbg_qT", [128, NH * S], F32,
                                            kind="ExternalOutput")
        dbg_exts["dbg_kT"] = nc.dram_tensor("dbg_kT", [128, NKV * S], F32,
                                            kind="ExternalOutput")
        dbg_exts["dbg_v"] = nc.dram_tensor("dbg_v", [128, NSC * NKV * 129],
                                           F32, kind="ExternalOutput")
        dbg_exts["dbg_oT"] = nc.dram_tensor("dbg_oT", [128, NH * S], F32,
                                            kind="ExternalOutput")

    with tile.TileContext(nc) as tc:
        cpool = tc.alloc_tile_pool(name="const", bufs=1)
        ppool = tc.alloc_tile_pool(name="persist", bufs=1)
        wpool = tc.alloc_tile_pool(name="wk", bufs=1)
        xpool = tc.alloc_tile_pool(name="xts", bufs=1)
        psW = tc.alloc_tile_pool(name="psW", bufs=5, space="PSUM")
        psO = tc.alloc_tile_pool(name="psO", bufs=1, space="PSUM")

        # ---- constants (engine setup ops are emitted after the startup
        # DMA issues so gpsimd's first weight DMAs trigger immediately) ----
        maskT = cpool.tile([128, 128], BF16)   # 1 where s1 >= s2 else 0
        ident_b = cpool.tile([128, 128], BF16)
        cd1_sb = cpool.tile([128, S], BF16)
        cd2_sb = cpool.tile([128, S], BF16)
        bias_sb = cpool.tile([128, 24], F32)

        # ---- persistent activations ----
        qT_all = ppool.tile([128, NH * S], BF16)    # [d, h*S + s]
        kT_all = ppool.tile([128, NKV * S], BF16)   # [d, g4*S + s]
        # v in [s2, d] layout + ones col for the denominator:
        # v_all[:, sc, g4, 0:128] = v chunk, [..., 128] = 1.0
        v_all = ppool.tile([128, NSC, NKV, 129], BF16)
        oT_all = ppool.tile([128, NH * S], BF16)    # [d, h*S + s]

        xts = [
            xpool.tile([128, S], BF16, tag=f"xt{dc}", name=f"xt{dc}")
            for dc in range(NDC)
        ]
        w0a = wpool.tile([128, 2048], BF16, tag="w", bufs=3, name="w_0a")
        w0b = wpool.tile([128, 2048], BF16, tag="w", bufs=3, name="w_0b")
        # startup: x streams as two half-column waves round-robin over the 3
        # DMA engines (arrival tracks the dc consumption order); block-0
        # weight pieces lead on sync/scalar/gpsimd at their consumption
        # deadlines, rope/bias constants fill in on scalar
        xq = [nc.sync, nc.scalar, nc.gpsimd]
        nc.sync.dma_start(out=w0a[:, 0:512], in_=wqkv_ext[0, :, 0:512])
        nc.scalar.dma_start(out=bias_sb[:], in_=bqkv_ext[:])
        nc.gpsimd.dma_start(out=w0a[:, 512:2048],
                            in_=wqkv_ext[0, :, 512:2048])
        nc.gpsimd.dma_start(out=w0b[:], in_=wqkv_ext[0, :, 2048:4096])
        for dc in range(NDC):
            xq[(dc + 1) % 3].dma_start(out=xts[dc][:, 0:512],
                                       in_=xt_ext[dc, :, 0:512])
            if dc == 10:
                nc.scalar.dma_start(out=cd1_sb[:], in_=cd1_ext[:])
            if dc == 13:
                nc.scalar.dma_start(out=cd2_sb[:], in_=cd2_ext[:])
        for dc in range(NDC):
            xq[(dc + 1) % 3].dma_start(out=xts[dc][:, 512:1024],
                                       in_=xt_ext[dc, :, 512:1024])

        # deferred constant setup (consumers start several blocks in, and
        # emitting these late keeps gpsimd's first weight DMAs unblocked)
        nc.gpsimd.memset(maskT[:], 1.0)
        nc.gpsimd.affine_select(
            out=maskT[:], in_=maskT[:], compare_op=mybir.AluOpType.is_ge,
            fill=0.0, base=0, pattern=[[1, 128]], channel_multiplier=-1)
        make_identity(nc, ident_b)
        nc.gpsimd.memset(v_all[:, :, :, 128:129], 1.0)

        w_tiles = {0: (w0a, w0b)}

        def ensure_w(bi):
            # issue block bi's weight DMAs one block ahead of use so the
            # sync queue's trigger is already in flight before any
            # attention-coupled wait (oT transpose) enters the sync stream
            if bi in w_tiles or bi >= 24:
                return
            w_a = wpool.tile([128, 2048], BF16, tag="w", bufs=3,
                             name=f"w{bi}a")
            nc.sync.dma_start(out=w_a[:], in_=wqkv_ext[bi, :, 0:2048])
            w_b = wpool.tile([128, 2048], BF16, tag="w", bufs=3,
                             name=f"w{bi}b")
            nc.gpsimd.dma_start(out=w_b[:], in_=wqkv_ext[bi, :, 2048:4096])
            w_tiles[bi] = (w_a, w_b)

        def emit_p1_block(bi, kind, idx, mid_cb=None):
            """Emits the block matmuls and the half-0 evict/rope chain.
            mid_cb (the previous head's av+normalize) is emitted at the
            middle of half 0 so its PE work hides inside the block and its
            DVE/queue consumers run mid-block instead of after it.
            Returns a closure emitting the half-1 rope tail (q/k blocks)
            or the second v transpose set, which the caller places where
            the PE has other work to hide the evict latency."""
            ensure_w(bi)
            w_a, w_b = w_tiles.pop(bi)
            ensure_w(bi + 1)
            acc0 = psW.tile([128, 512], F32, tag="acc", bufs=2,
                            name=f"acc0_{bi}")
            acc1 = psW.tile([128, 512], F32, tag="acc", bufs=2,
                            name=f"acc1_{bi}")
            tf = wpool.tile([128, S], BF16, tag="tf", bufs=2, name=f"tf{bi}")
            if kind != "v":
                dstT, col = (kT_all, idx) if kind == "k" else (qT_all, idx)
                t1 = wpool.tile([128, S], BF16, tag="rt0", bufs=2,
                                name=f"rt0_{bi}")

            def evict(hf, acc):
                nc.vector.tensor_scalar(
                    out=tf[:, hf * 512:(hf + 1) * 512], in0=acc[:],
                    scalar1=bias_sb[:, bi:bi + 1], scalar2=None,
                    op0=mybir.AluOpType.add)

            def rope(hf):
                # complex multiply via partition-offset DVE ops: the
                # half-swap writes the product of tf's half and cd2s (cd2
                # with halves pre-swapped on the host, so both INPUTS share
                # a partition base as the ISA requires) to the opposite
                # 64-partition half of t2
                cs = slice(hf * 512, (hf + 1) * 512)
                t2 = wpool.tile([128, 512], BF16, tag="rt1", bufs=2,
                                name=f"rt1_{bi}_{hf}")
                nc.vector.tensor_tensor(
                    out=t1[:, cs], in0=tf[:, cs], in1=cd1_sb[:, cs],
                    op=mybir.AluOpType.mult)
                nc.vector.tensor_tensor(
                    out=t2[0:64, :], in0=tf[64:128, cs],
                    in1=cd2_sb[64:128, cs], op=mybir.AluOpType.mult)
                nc.vector.tensor_tensor(
                    out=t2[64:128, :], in0=tf[0:64, cs],
                    in1=cd2_sb[0:64, cs], op=mybir.AluOpType.mult)
                nc.vector.tensor_tensor(
                    out=dstT[:, col * S + hf * 512:col * S + (hf + 1) * 512],
                    in0=t1[:, cs], in1=t2[:], op=mybir.AluOpType.add)

            def transposes(scs):
                for sc in scs:
                    tp = psW.tile([128, 128], BF16, tag="sc", bufs=3,
                                  name=f"tp{bi}_{sc}")
                    nc.tensor.transpose(
                        tp[:], tf[:, sc * 128:(sc + 1) * 128], ident_b)
                    nc.scalar.copy(v_all[:, sc, idx, 0:128], tp[:])

            # half 0 accumulates fully first so its evict/rope chain runs
            # during half 1's matmuls
            def wslice(dc):
                t = w_a if dc < 16 else w_b
                return t[:, (dc % 16) * 128:(dc % 16 + 1) * 128]

            for dc in range(NDC):
                nc.tensor.matmul(acc0[:], wslice(dc), xts[dc][:, 0:512],
                                 start=(dc == 0), stop=(dc == NDC - 1))
                if dc == 12 and mid_cb is not None:
                    mid_cb()
            evict(0, acc0)
            for dc in range(NDC):
                nc.tensor.matmul(acc1[:], wslice(dc), xts[dc][:, 512:1024],
                                 start=(dc == 0), stop=(dc == NDC - 1))
                if dc == 6 and kind != "v":
                    rope(0)
                if dc == 20 and kind == "v":
                    transposes(range(4))
            evict(1, acc1)
            if kind == "v":
                return lambda: transposes(range(4, NSC))
            return lambda: rope(1)

        def emit_scores_exps(h):
            """scoresT_j = kT_j.T @ qT per 512-chunk (half-0 column chunks
            first: their qT rope half finishes earlier); exp -> aT; causal
            mask on the diagonal 128-block (gpsimd). Returns
            {(j, ci): (aT tile, c0)}."""
            g4 = h // 4
            chunks = ([(j, 0) for j in range(4)] + [(j, 1) for j in range(4)]
                      + [(j, 0) for j in range(4, NSC)])
            aT = {}
            for j, ci in chunks:
                c0, c1 = _score_chunks(j)[ci]
                w = c1 - c0
                scp = psW.tile([128, w], F32, tag="sc", bufs=3,
                               name=f"scp{h}_{j}_{ci}")
                nc.tensor.matmul(
                    scp[:], kT_all[:, g4 * S + j * 128:g4 * S + (j + 1) * 128],
                    qT_all[:, h * S + c0:h * S + c1],
                    start=True, stop=True)
                a = wpool.tile([128, w], BF16, tag="aT", bufs=12,
                               name=f"aT{h}_{j}_{ci}")
                nc.scalar.activation(
                    a[:], scp[:], mybir.ActivationFunctionType.Exp,
                    scale=QK_SCALE)
                if ci == 0:
                    # causal mask on diagonal block (local cols 0:128)
                    nc.gpsimd.tensor_tensor(
                        out=a[:, 0:128], in0=a[:, 0:128], in1=maskT[:],
                        op=mybir.AluOpType.mult)
                aT[(j, ci)] = (a, c0)
            return aT

        def emit_av(h, aT, pe_transpose=False):
            """o[s1c, d|den] += aT_j[:, s1c].T @ [v_j | ones] — the 129th
            moving column lands the softmax denominator in the same psum
            accumulation group. Normalize per chunk (reciprocal +
            tensor_scalar), then transpose into oT_all (XBAR dma; PE
            matmul-transpose for the last heads so P3 isn't XBAR-blocked)."""
            g4 = h // 4
            # 8 groups of 129 f32, packed 3 per psum bank (3*129 <= 512)
            o_ps = psO.tile([128, 3, 512], F32, tag="o", bufs=1,
                            name=f"o_{h}")
            rcp = wpool.tile([128, NSC], F32, tag="rcp", bufs=2,
                             name=f"rcp{h}")
            o_sb = wpool.tile([128, NSC, 128], BF16, tag="osb", bufs=2,
                              name=f"osb{h}")
            def normalize(s1cs):
                # runs once a bank's groups all closed, so the DVE never
                # reads a psum bank the PE is still accumulating into
                for s1c in s1cs:
                    bank, off = divmod(s1c, 3)
                    off *= 129
                    nc.vector.reciprocal(rcp[:, s1c:s1c + 1],
                                         o_ps[:, bank, off + 128:off + 129])
                    nc.vector.tensor_scalar(
                        out=o_sb[:, s1c, :],
                        in0=o_ps[:, bank, off:off + 128],
                        scalar1=rcp[:, s1c:s1c + 1], scalar2=None,
                        op0=mybir.AluOpType.mult)
                    if pe_transpose:
                        dst = oT_all[:, h * S + s1c * 128:
                                     h * S + (s1c + 1) * 128]
                        tp = psW.tile([128, 128], BF16, tag="sc", bufs=3,
                                      name=f"tp_o{h}_{s1c}")
                        nc.tensor.transpose(tp[:], o_sb[:, s1c, :], ident_b)
                        nc.scalar.copy(dst, tp[:])

            def flush_oT():
                # XBAR transposes split across the two HWDGE queues; the
                # caller emits this AFTER the next head's exps so neither
                # queue has latency-critical work sitting behind the waits
                for s1c in range(NSC):
                    dst = oT_all[:, h * S + s1c * 128:h * S + (s1c + 1) * 128]
                    eng = nc.sync if s1c % 2 else nc.scalar
                    eng.dma_start(out=dst, in_=o_sb[:, s1c, :],
                                  transpose=True)

            for s1c in range(NSC):
                bank, off = divmod(s1c, 3)
                off *= 129
                for j in range(s1c + 1):
                    # find chunk holding global col s1c*128
                    if j < 4 and s1c >= 4:
                        a, c0 = aT[(j, 1)]
                    else:
                        a, c0 = aT[(j, 0)]
                    loc = s1c * 128 - c0
                    nc.tensor.matmul(
                        o_ps[:, bank, off:off + 129], a[:, loc:loc + 128],
                        v_all[:, j, g4, :],
                        start=(j == 0), stop=(j == s1c),
                        skip_group_check=True)
                if s1c == 2:
                    normalize((0, 1, 2))
                elif s1c == 5:
                    normalize((3, 4, 5))
            normalize((6, 7))
            return None if pe_transpose else flush_oT

        def fetch_wo(m):
            wo_m = wpool.tile([128, NH, 128], BF16, tag="wo", bufs=3,
                              name=f"wo{m}")
            nc.gpsimd.dma_start(out=wo_m[:, 0:8, :], in_=wo_ext[m, :, 0:1024])
            nc.scalar.dma_start(out=wo_m[:, 8:16, :],
                                in_=wo_ext[m, :, 1024:2048])
            return wo_m

        # ---------------- P1 + P2 interleaved ----------------
        state = {"pending": None, "flush": None}
        deferred = None  # half-1 tail of the previous block

        def mid_cb():
            # previous head's av, interleaved mid-block
            if state["pending"] is not None:
                h = state["pending"][0]
                state["flush"] = emit_av(*state["pending"],
                                         pe_transpose=(h >= NH - 2))
                state["pending"] = None

        for bi, (kind, idx) in enumerate(_block_specs()):
            tail1 = emit_p1_block(bi, kind, idx, None)
            mid_cb()
            if deferred is not None:
                deferred()
                deferred = None
            if kind == "q":
                tail1()  # q half-1 rope must precede its own scores
                state["pending"] = (idx, emit_scores_exps(idx))
            else:
                deferred = tail1
            if state["flush"] is not None:
                state["flush"]()
                state["flush"] = None
        wo_tiles = {m: fetch_wo(m) for m in range(3)}

        # ---------------- P3: Wo ----------------
        # P3 accumulators live in the P1 "acc" psum ring (no release
        # barrier); m=0's first 15 head-chunks are emitted BEFORE the last
        # head's av so the PE chews them while scalar runs head 15's exps.
        def p3_setup(m):
            wo_m = wo_tiles.pop(m) if m in wo_tiles else fetch_wo(m)
            acc_lo = psW.tile([128, 512], F32, tag="acc", bufs=2,
                              name=f"wp{m}_lo")
            acc_hi = psW.tile([128, 512], F32, tag="acc", bufs=2,
                              name=f"wp{m}_hi")
            return wo_m, acc_lo, acc_hi

        def p3_mms(st, c_from, c_to):
            wo_m, acc_lo, acc_hi = st
            for c in range(c_from, c_to):
                lhs = wo_m[:, c, :]
                nc.tensor.matmul(acc_lo[:], lhs,
                                 oT_all[:, c * S:c * S + 512],
                                 start=(c == 0), stop=(c == NH - 1))
                nc.tensor.matmul(acc_hi[:], lhs,
                                 oT_all[:, c * S + 512:c * S + 1024],
                                 start=(c == 0), stop=(c == NH - 1))

        def p3_finish(m, st):
            _, acc_lo, acc_hi = st
            oev = wpool.tile([128, S], BF16, tag="oev", bufs=2,
                             name=f"oev{m}")
            nc.scalar.copy(oev[:, 0:512], acc_lo[:])
            nc.scalar.copy(oev[:, 512:1024], acc_hi[:])
            nc.sync.dma_start(
                out=out_ext[m * 128:(m + 1) * 128, :], in_=oev[:])

        st0 = p3_setup(0)
        p3_mms(st0, 0, NH - 1)
        emit_av(*state["pending"], pe_transpose=True)
        p3_mms(st0, NH - 1, NH)
        p3_finish(0, st0)
        # m=1 also rides the acc ring so the pool-release barrier below
        # resolves while its c-loop runs
        st1 = p3_setup(1)
        p3_mms(st1, 0, NH)
        p3_finish(1, st1)

        psO.release()
        psW.release()
        xpool.release()
        psP3 = tc.alloc_tile_pool(name="psP3", bufs=2, space="PSUM")
        for m in range(2, NDC):
            wo_m = wo_tiles.pop(m) if m in wo_tiles else fetch_wo(m)
            acc = psP3.tile([128, S], F32, tag="wps", bufs=2, name=f"wp{m}")
            oev = wpool.tile([128, S], BF16, tag="oev", bufs=2,
                             name=f"oev{m}")
            if m < NDC - 1:
                for c in range(NH):
                    lhs = wo_m[:, c, :]
                    nc.tensor.matmul(acc[:, 0:512], lhs,
                                     oT_all[:, c * S:c * S + 512],
                                     start=(c == 0), stop=(c == NH - 1))
                    nc.tensor.matmul(acc[:, 512:1024], lhs,
                                     oT_all[:, c * S + 512:c * S + 1024],
                                     start=(c == 0), stop=(c == NH - 1))
                nc.scalar.copy(oev[:], acc[:])
                nc.sync.dma_start(
                    out=out_ext[m * 128:(m + 1) * 128, :], in_=oev[:])
            else:
                # last m-chunk: finish + ship the lo half while the PE runs
                # the hi half so the final out DMA overlaps compute
                for half in range(2):
                    cs = slice(half * 512, (half + 1) * 512)
                    for c in range(NH):
                        nc.tensor.matmul(acc[:, cs], wo_m[:, c, :],
                                         oT_all[:, c * S + half * 512:
                                                c * S + (half + 1) * 512],
                                         start=(c == 0), stop=(c == NH - 1))
                    nc.scalar.copy(oev[:, cs], acc[:, cs])
                    nc.sync.dma_start(
                        out=out_ext[m * 128:(m + 1) * 128,
                                    half * 512:(half + 1) * 512],
                        in_=oev[:, cs])
        psP3.release()

        if debug:
            for nm, t in [("dbg_qT", qT_all), ("dbg_kT", kT_all),
                          ("dbg_v", v_all), ("dbg_oT", oT_all)]:
                nc.gpsimd.dma_start(out=dbg_exts[nm][:], in_=t[:])

        cpool.seal()
        ppool.seal()
        wpool.seal()

    nc.compile()
    return nc


def _ev(base):
    return np.concatenate([np.arange(base, base + HD, 2),
                           np.arange(base + 1, base + HD, 2)])


def _pack_wblock(Wcols):
    # [D, 128] -> [128, NDC*128] with dc-major columns
    return Wcols.reshape(NDC, 128, 128).transpose(1, 0, 2).reshape(128, -1)


def kernel(x, freqs_cis, Wq, bq, Wk, bk, Wv, bv, Wo, bo, startpos):
    global LAST_PROFILE
    x = np.asarray(x, dtype=np.float32)
    freqs_cis = np.asarray(freqs_cis, dtype=np.float32)
    Wq = np.asarray(Wq, dtype=np.float32)
    Wk = np.asarray(Wk, dtype=np.float32)
    Wv = np.asarray(Wv, dtype=np.float32)
    Wo = np.asarray(Wo, dtype=np.float32)
    bq = np.asarray(bq, dtype=np.float32)
    bk = np.asarray(bk, dtype=np.float32)
    bv = np.asarray(bv, dtype=np.float32)
    bo = np.asarray(bo, dtype=np.float32)
    assert int(startpos) == 0

    bf = lambda a: np.ascontiguousarray(a.astype(ml_dtypes.bfloat16))
    f32c = lambda a: np.ascontiguousarray(a.astype(np.float32))

    # rope coefficients in [d, s] layout: C64[i, s] = fc[s, i, 0]
    C64 = freqs_cis[:, :, 0].T          # [64, S]
    D64 = freqs_cis[:, :, 1].T
    cd1 = bf(np.vstack([C64, C64]))
    # halves pre-swapped: row i<64 multiplies tf's half a (writing half b),
    # row i>=64 multiplies tf's half b (writing half a)
    cd2 = bf(np.vstack([D64, -D64]))

    in_maps = []
    for core in range(8):
        b, g = core // 2, core % 2
        if core < 2:  # weight shards depend only on g; reuse for later cores
            wblocks, bcols = [], []
            for kind, idx in _block_specs():
                if kind == "k":
                    sel = _ev((g * NKV + idx) * HD)
                    wblocks.append(_pack_wblock(Wk[:, sel]))
                    bcols.append(bk[sel])
                elif kind == "v":
                    base = (g * NKV + idx) * HD
                    sel = np.arange(base, base + HD)
                    wblocks.append(_pack_wblock(Wv[:, sel]))
                    bcols.append(bv[sel])
                else:
                    sel = _ev((g * NH + idx) * HD)
                    wblocks.append(_pack_wblock(Wq[:, sel]))
                    bcols.append(bq[sel])
            wqkv_h = bf(np.stack(wblocks))                  # [24, 128, 4096]
            bqkv = f32c(np.stack(bcols, axis=1))            # [128, 24]
            Wos = Wo[g * DQ:(g + 1) * DQ, :]                # [2048, 4096]
            wo_h = bf(np.stack([
                Wos[:, m * 128:(m + 1) * 128]
                .reshape(NH, 128, 128).transpose(1, 0, 2).reshape(128, -1)
                for m in range(NDC)
            ]))                                             # [32, 128, 2048]
        else:
            prev = in_maps[core - 2]
            wqkv_h, wo_h, bqkv = prev["wqkv"], prev["wo"], prev["bqkv"]
        xt_h = bf(x[b].T.reshape(NDC, 128, S))
        in_maps.append({
            "xt": xt_h, "wqkv": wqkv_h, "wo": wo_h,
            "cd1": cd1, "cd2": cd2, "bqkv": bqkv,
        })

    if "nc" not in _GRAPH_CACHE:
        _GRAPH_CACHE["nc"] = _build_graph()
    nc = _GRAPH_CACHE["nc"]

    res = run_bass_kernel_spmd(
        nc, in_maps, core_ids=list(range(8)),
        trace=bool(os.environ.get("BASS_TRACE")))
    LAST_PROFILE = res

    out = np.empty((B, S, D), dtype=np.float32)
    for b in range(B):
        t = (res.results[2 * b]["out"].astype(np.float32)
             + res.results[2 * b + 1]["out"].astype(np.float32))
        out[b] = t.T + bo[None, :]
    return out


# revision 32
# speedup vs baseline: 1.0585x; 1.0408x over previous
"""Trainium2 Bass kernel for GQA attention (B=4, S=1024, D=4096, HQ=32, HKV=8).

Sharding: 8 cores = 4 batches x 2 head-groups. Each core computes one batch
with 16 q-heads / 4 kv-heads (Wq/Wk/Wv column-sharded, Wo row-sharded). The
two head-group partial outputs per batch (bf16) are summed on the host, then
transposed (device emits out^T [Dout, S]) and bias bo added.

Device dataflow per core (all matmuls bf16):
  P1 (QKV) runs as 24 single-block column matmuls ordered [k,v,q0..q3] per
  kv group so attention for head h starts as soon as its q block is done.
  x streams in as two half-column waves over 4 DMA queues; w0/cd/bias ride
  the tensor-engine queue so the HBM-bound startup has no queue conflicts.
  P2 (attention) is interleaved into P1: per q block, emit scores
  (kT-stationary, already-transposed scoresT), exp on ScalarE (unsafe
  softmax), causal mask on the diagonal, then the PREVIOUS head's
  av+normalize so the scalar-engine exp of head h overlaps the PE work of
  head h+1 and the remaining P1 blocks. av uses aT-stationary matmuls with
  a 129-wide moving operand [v | ones] so the softmax denominator lands in
  psum column 128 of the same accumulation group (no separate den matmuls).
  Normalization is a per-chunk reciprocal + tensor_scalar multiply; oT for
  P3 is produced by XBAR dma transposes (gpsimd queue), except the last two
  heads which use PE transposes so P3's tail isn't blocked on the XBAR.
  The rope half-swap runs as partition-offset DVE multiplies (no PE time);
  v is transposed into [s2, d] layout by XBAR dma transposes.
  P3 (Wo): out[m-chunk, s] accumulated over 16 head-chunks with
  Wo-row-stationary matmuls; Wo streamed as 2x half DMAs per m-chunk on
  the gpsimd+scalar queues; per-block qkv weights alternate sync/vector
  queues. Output is written bf16 (host sums partials in f32); the last
  m-chunk evicts lo/hi halves sequentially so the final DMA overlaps PE.
"""

import math
import os

import numpy as np
import ml_dtypes

import concourse.bass as bass
import concourse.mybir as mybir
import concourse.tile as tile
from concourse import bacc
from concourse.bass_utils import run_bass_kernel_spmd
from concourse.masks import make_identity

B, S, D = 4, 1024, 4096
HQ, HKV, HD = 32, 8, 128
NH = 16          # q heads per core
NKV = 4          # kv heads per core
DQ = NH * HD     # 2048
DK = NKV * HD    # 512
NDC = D // 128   # 32 D-chunks
NSC = S // 128   # 8 s-chunks
QK_SCALE = 1.0 / math.sqrt(HD)

F32 = mybir.dt.float32
BF16 = mybir.dt.bfloat16

_GRAPH_CACHE = {}
LAST_PROFILE = None


def _block_specs():
    """24 P1 blocks in emission order: per kv group [k, v, q0..q3]."""
    blocks = []
    for g4 in range(NKV):
        blocks.append(("k", g4))
        blocks.append(("v", g4))
        for i in range(4):
            blocks.append(("q", 4 * g4 + i))
    return blocks


def _score_chunks(j):
    """Global (c0, c1) column chunks for k-chunk j (causal: s1 >= j*128),
    split at the 512 boundary so each chunk fits one psum bank."""
    if j < 4:
        return [(j * 128, 512), (512, 1024)]
    return [(j * 128, 1024)]


def _build_graph():
    nc = bacc.Bacc(debug=False)

    xt_ext = nc.dram_tensor("xt", [NDC, 128, S], BF16, kind="ExternalInput")
    # per-block weight columns, dc-major rows: [24, 128, NDC*128]
    wqkv_ext = nc.dram_tensor("wqkv", [24, 128, NDC * 128], BF16,
                              kind="ExternalInput")
    # Wo row-shard packed per output m-chunk: [32, 128, NH*128]
    wo_ext = nc.dram_tensor("wo", [NDC, 128, NH * 128], BF16,
                            kind="ExternalInput")
    cd1_ext = nc.dram_tensor("cd1", [128, S], BF16, kind="ExternalInput")
    cd2_ext = nc.dram_tensor("cd2", [128, S], BF16, kind="ExternalInput")
    # bias column tile: col bi = bias for block bi
    bqkv_ext = nc.dram_tensor("bqkv", [128, 24], F32, kind="ExternalInput")
    out_ext = nc.dram_tensor("out", [D, S], BF16, kind="ExternalOutput")
    debug = bool(os.environ.get("BASS_DEBUG_TAPS"))
    dbg_exts = {}
    if debug:
        dbg_exts["dbg_qT"] = nc.dram_tensor("dbg_qT", [128, NH * S], F32,
                                            kind="ExternalOutput")
        dbg_exts["dbg_kT"] = nc.dram_tensor("dbg_kT", [128, NKV * S], F32,
                                            kind="ExternalOutput")
        dbg_exts["dbg_v"] = nc.dram_tensor("dbg_v", [128, NSC * NKV * 129],
                                           F32, kind="ExternalOutput")
        dbg_exts["dbg_oT"] = nc.dram_tensor("dbg_oT", [128, NH * S], F32,
                                            kind="ExternalOutput")

    with tile.TileContext(nc) as tc:
        cpool = tc.alloc_tile_pool(name="const", bufs=1)
        ppool = tc.alloc_tile_pool(name="persist", bufs=1)
        wpool = tc.alloc_tile_pool(name="wk", bufs=1)
        xpool = tc.alloc_tile_pool(name="xts", bufs=1)
        psW = tc.alloc_tile_pool(name="psW", bufs=5, space="PSUM")
        psO = tc.alloc_tile_pool(name="psO", bufs=1, space="PSUM")

        # ---- constants (engine setup ops are emitted after the startup
        # DMA issues so gpsimd's first weight DMAs trigger immediately) ----
        maskT = cpool.tile([128, 128], BF16)   # 1 where s1 >= s2 else 0
        ident_b = cpool.tile([128, 128], BF16)
        cd1_sb = cpool.tile([128, S], BF16)
        cd2_sb = cpool.tile([128, S], BF16)
        bias_sb = cpool.tile([128, 24], F32)

        # ---- persistent activations ----
        qT_all = ppool.tile([128, NH * S], BF16)    # [d, h*S + s]
        kT_all = ppool.tile([128, NKV * S], BF16)   # [d, g4*S + s]
        # v in [s2, d] layout + ones col for the denominator:
        # v_all[:, sc, g4, 0:128] = v chunk, [..., 128] = 1.0
        v_all = ppool.tile([128, NSC, NKV, 129], BF16)
        oT_all = ppool.tile([128, NH * S], BF16)    # [d, h*S + s]

        xts = [
            xpool.tile([128, S], BF16, tag=f"xt{dc}", name=f"xt{dc}")
            for dc in range(NDC)
        ]
        w0a = wpool.tile([128, 2048], BF16, tag="w", bufs=3, name="w_0a")
        w0b = wpool.tile([128, 2048], BF16, tag="w", bufs=3, name="w_0b")
        # startup: x streams as two half-column waves round-robin over the 3
        # DMA engines (arrival tracks the dc consumption order); block-0
        # weight pieces lead on sync/scalar/gpsimd at their consumption
        # deadlines, rope/bias constants fill in on scalar
        xq = [nc.sync, nc.scalar, nc.gpsimd]
        nc.sync.dma_start(out=w0a[:, 0:512], in_=wqkv_ext[0, :, 0:512])
        nc.scalar.dma_start(out=bias_sb[:], in_=bqkv_ext[:])
        nc.gpsimd.dma_start(out=w0a[:, 512:2048],
                            in_=wqkv_ext[0, :, 512:2048])
        nc.gpsimd.dma_start(out=w0b[:], in_=wqkv_ext[0, :, 2048:4096])
        for dc in range(NDC):
            xq[(dc + 1) % 3].dma_start(out=xts[dc][:, 0:512],
                                       in_=xt_ext[dc, :, 0:512])
            if dc == 10:
                nc.scalar.dma_start(out=cd1_sb[:], in_=cd1_ext[:])
            if dc == 13:
                nc.scalar.dma_start(out=cd2_sb[:], in_=cd2_ext[:])
        for dc in range(NDC):
            xq[(dc + 1) % 3].dma_start(out=xts[dc][:, 512:1024],
                                       in_=xt_ext[dc, :, 512:1024])

        # deferred constant setup (consumers start several blocks in, and
        # emitting these late keeps gpsimd's first weight DMAs unblocked)
        nc.gpsimd.memset(maskT[:], 1.0)
        nc.gpsimd.affine_select(
            out=maskT[:], in_=maskT[:], compare_op=mybir.AluOpType.is_ge,
            fill=0.0, base=0, pattern=[[1, 128]], channel_multiplier=-1)
        make_identity(nc, ident_b)
        nc.gpsimd.memset(v_all[:, :, :, 128:129], 1.0)

        w_tiles = {0: (w0a, w0b)}

        def ensure_w(bi):
            # issue block bi's weight DMAs one block ahead of use so the
            # sync queue's trigger is already in flight before any
            # attention-coupled wait (oT transpose) enters the sync stream
            if bi in w_tiles or bi >= 24:
                return
            w_a = wpool.tile([128, 2048], BF16, tag="w", bufs=3,
                             name=f"w{bi}a")
            nc.sync.dma_start(out=w_a[:], in_=wqkv_ext[bi, :, 0:2048])
            w_b = wpool.tile([128, 2048], BF16, tag="w", bufs=3,
                             name=f"w{bi}b")
            nc.gpsimd.dma_start(out=w_b[:], in_=wqkv_ext[bi, :, 2048:4096])
            w_tiles[bi] = (w_a, w_b)

        def emit_p1_block(bi, kind, idx, mid_cb=None):
            """Emits the block matmuls and the half-0 evict/rope chain.
            mid_cb (the previous head's av+normalize) is emitted at the
            middle of half 0 so its PE work hides inside the block and its
            DVE/queue consumers run mid-block instead of after it.
            Returns a closure emitting the half-1 rope tail (q/k blocks)
            or the second v transpose set, which the caller places where
            the PE has other work to hide the evict latency."""
            ensure_w(bi)
            w_a, w_b = w_tiles.pop(bi)
            ensure_w(bi + 1)
            acc0 = psW.tile([128, 512], F32, tag="acc", bufs=2,
                            name=f"acc0_{bi}")
            acc1 = psW.tile([128, 512], F32, tag="acc", bufs=2,
                            name=f"acc1_{bi}")
            tf = wpool.tile([128, S], BF16, tag="tf", bufs=2, name=f"tf{bi}")
            if kind != "v":
                dstT, col = (kT_all, idx) if kind == "k" else (qT_all, idx)
                t1 = wpool.tile([128, S], BF16, tag="rt0", bufs=2,
                                name=f"rt0_{bi}")

            def evict(hf, acc):
                nc.vector.tensor_scalar(
                    out=tf[:, hf * 512:(hf + 1) * 512], in0=acc[:],
                    scalar1=bias_sb[:, bi:bi + 1], scalar2=None,
                    op0=mybir.AluOpType.add)

            def rope(hf):
                # complex multiply via partition-offset DVE ops: the
                # half-swap writes the product of tf's half and cd2s (cd2
                # with halves pre-swapped on the host, so both INPUTS share
                # a partition base as the ISA requires) to the opposite
                # 64-partition half of t2
                cs = slice(hf * 512, (hf + 1) * 512)
                t2 = wpool.tile([128, 512], BF16, tag="rt1", bufs=2,
                                name=f"rt1_{bi}_{hf}")
                nc.vector.tensor_tensor(
                    out=t1[:, cs], in0=tf[:, cs], in1=cd1_sb[:, cs],
                    op=mybir.AluOpType.mult)
                nc.vector.tensor_tensor(
                    out=t2[0:64, :], in0=tf[64:128, cs],
                    in1=cd2_sb[64:128, cs], op=mybir.AluOpType.mult)
                nc.vector.tensor_tensor(
                    out=t2[64:128, :], in0=tf[0:64, cs],
                    in1=cd2_sb[0:64, cs], op=mybir.AluOpType.mult)
                nc.vector.tensor_tensor(
                    out=dstT[:, col * S + hf * 512:col * S + (hf + 1) * 512],
                    in0=t1[:, cs], in1=t2[:], op=mybir.AluOpType.add)

            def transposes(scs):
                for sc in scs:
                    tp = psW.tile([128, 128], BF16, tag="sc", bufs=3,
                                  name=f"tp{bi}_{sc}")
                    nc.tensor.transpose(
                        tp[:], tf[:, sc * 128:(sc + 1) * 128], ident_b)
                    nc.scalar.copy(v_all[:, sc, idx, 0:128], tp[:])

            # half 0 accumulates fully first so its evict/rope chain runs
            # during half 1's matmuls
            def wslice(dc):
                t = w_a if dc < 16 else w_b
                return t[:, (dc % 16) * 128:(dc % 16 + 1) * 128]

            for dc in range(NDC):
                nc.tensor.matmul(acc0[:], wslice(dc), xts[dc][:, 0:512],
                                 start=(dc == 0), stop=(dc == NDC - 1))
                if dc == 12 and mid_cb is not None:
                    mid_cb()
            evict(0, acc0)
            for dc in range(NDC):
                nc.tensor.matmul(acc1[:], wslice(dc), xts[dc][:, 512:1024],
                                 start=(dc == 0), stop=(dc == NDC - 1))
                if dc == 6 and kind != "v":
                    rope(0)
                if dc == 20 and kind == "v":
                    transposes(range(4))
            evict(1, acc1)
            if kind == "v":
                return lambda: transposes(range(4, NSC))
            return lambda: rope(1)

        def emit_scores_exps(h):
            """scoresT_j = kT_j.T @ qT per 512-chunk (half-0 column chunks
            first: their qT rope half finishes earlier); exp -> aT; causal
            mask on the diagonal 128-block (gpsimd). Returns
            {(j, ci): (aT tile, c0)}."""
            g4 = h // 4
            chunks = ([(j, 0) for j in range(4)] + [(j, 1) for j in range(4)]
                      + [(j, 0) for j in range(4, NSC)])
            aT = {}
            for j, ci in chunks:
                c0, c1 = _score_chunks(j)[ci]
                w = c1 - c0
                scp = psW.tile([128, w], F32, tag="sc", bufs=3,
                               name=f"scp{h}_{j}_{ci}")
                nc.tensor.matmul(
                    scp[:], kT_all[:, g4 * S + j * 128:g4 * S + (j + 1) * 128],
                    qT_all[:, h * S + c0:h * S + c1],
                    start=True, stop=True)
                a = wpool.tile([128, w], BF16, tag="aT", bufs=12,
                               name=f"aT{h}_{j}_{ci}")
                nc.scalar.activation(
                    a[:], scp[:], mybir.ActivationFunctionType.Exp,
                    scale=QK_SCALE)
                if ci == 0:
                    # causal mask on diagonal block (local cols 0:128)
                    nc.gpsimd.tensor_tensor(
                        out=a[:, 0:128], in0=a[:, 0:128], in1=maskT[:],
                        op=mybir.AluOpType.mult)
                aT[(j, ci)] = (a, c0)
            return aT

        def emit_av(h, aT, pe_transpose=False):
            """o[s1c, d|den] += aT_j[:, s1c].T @ [v_j | ones] — the 129th
            moving column lands the softmax denominator in the same psum
            accumulation group. Normalize per chunk (reciprocal +
            tensor_scalar), then transpose into oT_all (XBAR dma; PE
            matmul-transpose for the last heads so P3 isn't XBAR-blocked)."""
            g4 = h // 4
            # 8 groups of 129 f32, packed 3 per psum bank (3*129 <= 512)
            o_ps = psO.tile([128, 3, 512], F32, tag="o", bufs=1,
                            name=f"o_{h}")
            rcp = wpool.tile([128, NSC], F32, tag="rcp", bufs=2,
                             name=f"rcp{h}")
            o_sb = wpool.tile([128, NSC, 128], BF16, tag="osb", bufs=2,
                              name=f"osb{h}")
            def normalize(s1cs):
                # runs once a bank's groups all closed, so the DVE never
                # reads a psum bank the PE is still accumulating into
                for s1c in s1cs:
                    bank, off = divmod(s1c, 3)
                    off *= 129
                    nc.vector.reciprocal(rcp[:, s1c:s1c + 1],
                                         o_ps[:, bank, off + 128:off + 129])
                    nc.vector.tensor_scalar(
                        out=o_sb[:, s1c, :],
                        in0=o_ps[:, bank, off:off + 128],
                        scalar1=rcp[:, s1c:s1c + 1], scalar2=None,
                        op0=mybir.AluOpType.mult)
                    dst = oT_all[:, h * S + s1c * 128:
                                 h * S + (s1c + 1) * 128]
                    if pe_transpose:
                        tp = psW.tile([128, 128], BF16, tag="sc", bufs=3,
                                      name=f"tp_o{h}_{s1c}")
                        nc.tensor.transpose(tp[:], o_sb[:, s1c, :], ident_b)
                        nc.scalar.copy(dst, tp[:])
                    else:
                        nc.sync.dma_start(out=dst, in_=o_sb[:, s1c, :],
                                          transpose=True)

            for s1c in range(NSC):
                bank, off = divmod(s1c, 3)
                off *= 129
                for j in range(s1c + 1):
                    # find chunk holding global col s1c*128
                    if j < 4 and s1c >= 4:
                        a, c0 = aT[(j, 1)]
                    else:
                        a, c0 = aT[(j, 0)]
                    loc = s1c * 128 - c0
                    nc.tensor.matmul(
                        o_ps[:, bank, off:off + 129], a[:, loc:loc + 128],
                        v_all[:, j, g4, :],
                        start=(j == 0), stop=(j == s1c),
                        skip_group_check=True)
                if s1c == 2:
                    normalize((0, 1, 2))
                elif s1c == 5:
                    normalize((3, 4, 5))
            normalize((6, 7))

        def fetch_wo(m):
            wo_m = wpool.tile([128, NH, 128], BF16, tag="wo", bufs=3,
                              name=f"wo{m}")
            nc.gpsimd.dma_start(out=wo_m[:, 0:8, :], in_=wo_ext[m, :, 0:1024])
            nc.scalar.dma_start(out=wo_m[:, 8:16, :],
                                in_=wo_ext[m, :, 1024:2048])
            return wo_m

        # ---------------- P1 + P2 interleaved ----------------
        state = {"pending": None}
        deferred = None  # half-1 tail of the previous block

        def mid_cb():
            # previous head's av, interleaved mid-block
            if state["pending"] is not None:
                h = state["pending"][0]
                emit_av(*state["pending"], pe_transpose=(h >= NH - 2))
                state["pending"] = None

        for bi, (kind, idx) in enumerate(_block_specs()):
            tail1 = emit_p1_block(bi, kind, idx, mid_cb)
            if deferred is not None:
                deferred()
                deferred = None
            if kind == "q":
                tail1()  # q half-1 rope must precede its own scores
                state["pending"] = (idx, emit_scores_exps(idx))
            else:
                deferred = tail1
        wo_tiles = {m: fetch_wo(m) for m in range(3)}

        # ---------------- P3: Wo ----------------
        # P3 accumulators live in the P1 "acc" psum ring (no release
        # barrier); m=0's first 15 head-chunks are emitted BEFORE the last
        # head's av so the PE chews them while scalar runs head 15's exps.
        def p3_setup(m):
            wo_m = wo_tiles.pop(m) if m in wo_tiles else fetch_wo(m)
            acc_lo = psW.tile([128, 512], F32, tag="acc", bufs=2,
                              name=f"wp{m}_lo")
            acc_hi = psW.tile([128, 512], F32, tag="acc", bufs=2,
                              name=f"wp{m}_hi")
            return wo_m, acc_lo, acc_hi

        def p3_mms(st, c_from, c_to):
            wo_m, acc_lo, acc_hi = st
            for c in range(c_from, c_to):
                lhs = wo_m[:, c, :]
                nc.tensor.matmul(acc_lo[:], lhs,
                                 oT_all[:, c * S:c * S + 512],
                                 start=(c == 0), stop=(c == NH - 1))
                nc.tensor.matmul(acc_hi[:], lhs,
                                 oT_all[:, c * S + 512:c * S + 1024],
                                 start=(c == 0), stop=(c == NH - 1))

        def p3_finish(m, st):
            _, acc_lo, acc_hi = st
            oev = wpool.tile([128, S], BF16, tag="oev", bufs=2,
                             name=f"oev{m}")
            nc.scalar.copy(oev[:, 0:512], acc_lo[:])
            nc.scalar.copy(oev[:, 512:1024], acc_hi[:])
            nc.sync.dma_start(
                out=out_ext[m * 128:(m + 1) * 128, :], in_=oev[:])

        st0 = p3_setup(0)
        p3_mms(st0, 0, NH - 1)
        emit_av(*state["pending"], pe_transpose=True)
        p3_mms(st0, NH - 1, NH)
        p3_finish(0, st0)
        # m=1 also rides the acc ring so the pool-release barrier below
        # resolves while its c-loop runs
        st1 = p3_setup(1)
        p3_mms(st1, 0, NH)
        p3_finish(1, st1)

        psO.release()
        psW.release()
        xpool.release()
        psP3 = tc.alloc_tile_pool(name="psP3", bufs=2, space="PSUM")
        for m in range(2, NDC):
            wo_m = wo_tiles.pop(m) if m in wo_tiles else fetch_wo(m)
            acc = psP3.tile([128, S], F32, tag="wps", bufs=2, name=f"wp{m}")
            oev = wpool.tile([128, S], BF16, tag="oev", bufs=2,
                             name=f"oev{m}")
            if m < NDC - 1:
                for c in range(NH):
                    lhs = wo_m[:, c, :]
                    nc.tensor.matmul(acc[:, 0:512], lhs,
                                     oT_all[:, c * S:c * S + 512],
                                     start=(c == 0), stop=(c == NH - 1))
                    nc.tensor.matmul(acc[:, 512:1024], lhs,
                                     oT_all[:, c * S + 512:c * S + 1024],
                                     start=(c == 0), stop=(c == NH - 1))
                nc.scalar.copy(oev[:], acc[:])
                nc.sync.dma_start(
                    out=out_ext[m * 128:(m + 1) * 128, :], in_=oev[:])
            else:
                # last m-chunk: finish + ship the lo half while the PE runs
                # the hi half so the final out DMA overlaps compute
                for half in range(2):
                    cs = slice(half * 512, (half + 1) * 512)
                    for c in range(NH):
                        nc.tensor.matmul(acc[:, cs], wo_m[:, c, :],
                                         oT_all[:, c * S + half * 512:
                                                c * S + (half + 1) * 512],
                                         start=(c == 0), stop=(c == NH - 1))
                    nc.scalar.copy(oev[:, cs], acc[:, cs])
                    nc.sync.dma_start(
                        out=out_ext[m * 128:(m + 1) * 128,
                                    half * 512:(half + 1) * 512],
                        in_=oev[:, cs])
        psP3.release()

        if debug:
            for nm, t in [("dbg_qT", qT_all), ("dbg_kT", kT_all),
                          ("dbg_v", v_all), ("dbg_oT", oT_all)]:
                nc.gpsimd.dma_start(out=dbg_exts[nm][:], in_=t[:])

        cpool.seal()
        ppool.seal()
        wpool.seal()

    nc.compile()
    return nc


def _ev(base):
    return np.concatenate([np.arange(base, base + HD, 2),
                           np.arange(base + 1, base + HD, 2)])


def _pack_wblock(Wcols):
    # [D, 128] -> [128, NDC*128] with dc-major columns
    return Wcols.reshape(NDC, 128, 128).transpose(1, 0, 2).reshape(128, -1)


def kernel(x, freqs_cis, Wq, bq, Wk, bk, Wv, bv, Wo, bo, startpos):
    global LAST_PROFILE
    x = np.asarray(x, dtype=np.float32)
    freqs_cis = np.asarray(freqs_cis, dtype=np.float32)
    Wq = np.asarray(Wq, dtype=np.float32)
    Wk = np.asarray(Wk, dtype=np.float32)
    Wv = np.asarray(Wv, dtype=np.float32)
    Wo = np.asarray(Wo, dtype=np.float32)
    bq = np.asarray(bq, dtype=np.float32)
    bk = np.asarray(bk, dtype=np.float32)
    bv = np.asarray(bv, dtype=np.float32)
    bo = np.asarray(bo, dtype=np.float32)
    assert int(startpos) == 0

    bf = lambda a: np.ascontiguousarray(a.astype(ml_dtypes.bfloat16))
    f32c = lambda a: np.ascontiguousarray(a.astype(np.float32))

    # rope coefficients in [d, s] layout: C64[i, s] = fc[s, i, 0]
    C64 = freqs_cis[:, :, 0].T          # [64, S]
    D64 = freqs_cis[:, :, 1].T
    cd1 = bf(np.vstack([C64, C64]))
    # halves pre-swapped: row i<64 multiplies tf's half a (writing half b),
    # row i>=64 multiplies tf's half b (writing half a)
    cd2 = bf(np.vstack([D64, -D64]))

    in_maps = []
    for core in range(8):
        b, g = core // 2, core % 2
        if core < 2:  # weight shards depend only on g; reuse for later cores
            wblocks, bcols = [], []
            for kind, idx in _block_specs():
                if kind == "k":
                    sel = _ev((g * NKV + idx) * HD)
                    wblocks.append(_pack_wblock(Wk[:, sel]))
                    bcols.append(bk[sel])
                elif kind == "v":
                    base = (g * NKV + idx) * HD
                    sel = np.arange(base, base + HD)
                    wblocks.append(_pack_wblock(Wv[:, sel]))
                    bcols.append(bv[sel])
                else:
                    sel = _ev((g * NH + idx) * HD)
                    wblocks.append(_pack_wblock(Wq[:, sel]))
                    bcols.append(bq[sel])
            wqkv_h = bf(np.stack(wblocks))                  # [24, 128, 4096]
            bqkv = f32c(np.stack(bcols, axis=1))            # [128, 24]
            Wos = Wo[g * DQ:(g + 1) * DQ, :]                # [2048, 4096]
            wo_h = bf(np.stack([
                Wos[:, m * 128:(m + 1) * 128]
                .reshape(NH, 128, 128).transpose(1, 0, 2).reshape(128, -1)
                for m in range(NDC)
            ]))                                             # [32, 128, 2048]
        else:
            prev = in_maps[core - 2]
            wqkv_h, wo_h, bqkv = prev["wqkv"], prev["wo"], prev["bqkv"]
        xt_h = bf(x[b].T.reshape(NDC, 128, S))
        in_maps.append({
            "xt": xt_h, "wqkv": wqkv_h, "wo": wo_h,
            "cd1": cd1, "cd2": cd2, "bqkv": bqkv,
        })

    if "nc" not in _GRAPH_CACHE:
        _GRAPH_CACHE["nc"] = _build_graph()
    nc = _GRAPH_CACHE["nc"]

    res = run_bass_kernel_spmd(
        nc, in_maps, core_ids=list(range(8)),
        trace=bool(os.environ.get("BASS_TRACE")))
    LAST_PROFILE = res

    out = np.empty((B, S, D), dtype=np.float32)
    for b in range(B):
        t = (res.results[2 * b]["out"].astype(np.float32)
             + res.results[2 * b + 1]["out"].astype(np.float32))
        out[b] = t.T + bo[None, :]
    return out
